# revision 2
# baseline (speedup 1.0000x reference)
"""Multi-head causal attention with RoPE (B=1, S=4096, D=1024, H=16) on 8
Trainium2 NeuronCores.

Sharding: tensor-parallel over heads — each core computes 2 heads (QKV
projections column-sliced, attention, and its rank-128 partial of the output
projection). The 8 partial outputs are summed on the host (row-parallel wo).

Layout strategy: everything on-chip is kept transposed ([feature, seq]) so
the PE systolic array contracts over the partition dim at every stage with
no on-chip transposes of activations:
  - x.T uploaded host-side; q.T/k.T/v.T = W_slice @ x.T
  - RoPE via a second projection with host-swapped/negated weight rows:
    rot = (W q)∘cosP + (W_swap q)∘sinP  (pure elementwise, no partition shifts)
  - scores.T[sk,sq] = k.T_blk.T? -> matmul(lhsT=kT[64,128], rhs=qT[64,512])
  - softmax: exp on ACT (scale=1/8 folded in); denominators via an extra
    ones-column appended to V (row 64 of the attn@V accumulator)
  - attn@V: matmul(lhsT=[V|1][128sk,65], rhs=exp.T[128sk,512]) accumulated
    over sk tiles -> out.T[65,512]
  - O-proj: matmul(lhsT=wo_slice.T[128g,128dout], rhs=attn.T) -> partial.T
Causality: above-diagonal 128x512 score tiles are skipped; the 4
diagonal-straddling tiles per 512-chunk add real mask values (pre-scaled x8
so the 1/8 exp scale reproduces additive mask semantics exactly).
"""
import math
import numpy as np

import concourse.bass as bass
import concourse.mybir as mybir
import concourse.tile as tile
from concourse.bass_utils import run_bass_kernel_spmd

B, S, D, H = 1, 4096, 1024, 16
HD = D // H            # 64
NC = 8                 # cores
HPC = H // NC          # 2 heads per core
SQC = 512              # seq chunk (matmul free dim)
NJ = S // SQC          # 8 chunks
NKT = S // 128         # 32 sk partition tiles
KT = D // 128          # 8 contraction tiles for projections

F32 = mybir.dt.float32
AF = mybir.ActivationFunctionType

_MAX_WAITS = 1


def _fix_waits(nc):
    """walrus in this container rejects >1 sync-wait on one instruction
    ("Too many sync wait commands"). Split excess waits onto preceding
    same-engine NoOps (semantics preserved: engine blocks in order)."""
    n = 0
    for fn in nc.m.functions:
        for bb in fn.blocks:
            new_list = []
            for inst in bb.instructions:
                si = getattr(inst, "sync_info", None)
                if si is not None and si.on_wait and len(si.on_wait) > _MAX_WAITS:
                    waits = list(si.on_wait)
                    excess, keep = waits[:-_MAX_WAITS], waits[-_MAX_WAITS:]
                    for j in range(0, len(excess), _MAX_WAITS):
                        nop = mybir.InstNoOp(
                            name=f"I-waitfix-{nc.next_id()}",
                            ins=[],
                            outs=[],
                            engine=inst.engine,
                            sync_info=mybir.SyncInfo(
                                on_wait=excess[j : j + _MAX_WAITS], on_update=[]
                            ),
                        )
                        nc.register_instruction(nop)
                        new_list.append(nop)
                        n += 1
                    si.on_wait = keep
                new_list.append(inst)
            bb.instructions[:] = new_list
    return n


def build_program(mode: str, mm_dt=F32):
    """mode: 'causal' (skip above-diag tiles, mask on 4 diag tiles/chunk),
    'zeros' (no mask at all), 'general' (mask added on every tile)."""
    nc = bass.Bass()

    xT_d = nc.dram_tensor("xT", (D, S), mm_dt, kind="ExternalInput")
    w_d = {
        n: nc.dram_tensor(n, (D, 128), mm_dt, kind="ExternalInput")
        for n in ("wq", "wqs", "wk", "wks", "wv")
    }
    wo_d = nc.dram_tensor("wo", (128, D), mm_dt, kind="ExternalInput")
    cos_d = nc.dram_tensor("cosP", (128, S), F32, kind="ExternalInput")
    sin_d = nc.dram_tensor("sinP", (128, S), F32, kind="ExternalInput")
    eye_d = nc.dram_tensor("eye2", (128, 64), mm_dt, kind="ExternalInput")
    if mode == "causal":
        mask_d = nc.dram_tensor("maskd", (NJ, SQC, SQC), F32, kind="ExternalInput")
    elif mode == "general":
        mask_d = nc.dram_tensor("maskd", (NJ, S, SQC), F32, kind="ExternalInput")
    else:
        mask_d = None
    out_d = nc.dram_tensor("opT", (D, S), F32, kind="ExternalOutput")

    with tile.TileContext(nc) as tc:
        with (
            tc.tile_pool(name="wts", bufs=1) as wts,
            tc.tile_pool(name="big", bufs=1) as big,
            tc.tile_pool(name="xc", bufs=2) as xcp,
            tc.tile_pool(name="cs", bufs=2) as csp,
            tc.tile_pool(name="rp", bufs=2) as rpp,
            tc.tile_pool(name="ex", bufs=4) as exp_p,
            tc.tile_pool(name="mk", bufs=3) as mkp,
            tc.tile_pool(name="af", bufs=2) as afp,
            tc.tile_pool(name="tm", bufs=2) as tmp_p,
            tc.tile_pool(name="oo", bufs=3) as oop,
            tc.tile_pool(name="rc", bufs=2) as rcp,
            tc.tile_pool(name="bc", bufs=2) as bcp_p,
            tc.tile_pool(name="pp", bufs=2, space=bass.MemorySpace.PSUM) as ppp,
            tc.tile_pool(name="sc", bufs=3, space=bass.MemorySpace.PSUM) as scp,
            tc.tile_pool(name="at0", bufs=1, space=bass.MemorySpace.PSUM) as at0p,
            tc.tile_pool(name="at1", bufs=1, space=bass.MemorySpace.PSUM) as at1p,
        ):
            # ---- weights / constants in SBUF ----
            w_sb = {}
            for n in ("wq", "wqs", "wk", "wks", "wv"):
                t = wts.tile([128, KT, 128], mm_dt, tag=n)
                for k in range(KT):
                    nc.sync.dma_start(t[:, k, :], w_d[n][k * 128 : (k + 1) * 128, :])
                w_sb[n] = t
            wo_sb = wts.tile([128, D], mm_dt, tag="wo")
            nc.sync.dma_start(wo_sb[:], wo_d[:])
            eye_sb = wts.tile([128, 64], mm_dt, tag="eye")
            nc.sync.dma_start(eye_sb[:], eye_d[:])
            ones_sb = wts.tile([1, 64], mm_dt, tag="ones")
            nc.vector.memset(ones_sb[:], 1.0)

            qrot = big.tile([128, S], mm_dt, tag="qrot")
            krot = big.tile([128, S], mm_dt, tag="krot")
            vT = big.tile([128, S], F32, tag="vT")
            vext = big.tile([128, HPC * NKT * 65], mm_dt, tag="vext")

            # ---- phase A: projections + RoPE, chunk by chunk ----
            for j in range(NJ):
                sl = slice(j * SQC, (j + 1) * SQC)
                xc = xcp.tile([128, KT, SQC], mm_dt, tag="xc")
                for k in range(KT):
                    nc.sync.dma_start(
                        xc[:, k, :], xT_d[k * 128 : (k + 1) * 128, sl]
                    )
                cosc = csp.tile([128, SQC], F32, tag="cosc")
                sinc = csp.tile([128, SQC], F32, tag="sinc")
                nc.sync.dma_start(cosc[:], cos_d[:, sl])
                nc.sync.dma_start(sinc[:], sin_d[:, sl])

                def proj(wname):
                    ps = ppp.tile([128, SQC], F32, tag="pp")
                    for k in range(KT):
                        nc.tensor.matmul(
                            ps[:],
                            w_sb[wname][:, k, :],
                            xc[:, k, :],
                            start=(k == 0),
                            stop=(k == KT - 1),
                        )
                    return ps

                for main_w, swap_w, dest in (("wq", "wqs", qrot), ("wk", "wks", krot)):
                    ps_m = proj(main_w)
                    t1 = rpp.tile([128, SQC], F32, tag="t1")
                    nc.vector.tensor_mul(t1[:], ps_m[:], cosc[:])
                    ps_s = proj(swap_w)
                    t2 = rpp.tile([128, SQC], F32, tag="t2")
                    nc.vector.tensor_mul(t2[:], ps_s[:], sinc[:])
                    nc.vector.tensor_add(dest[:, sl], t1[:], t2[:])
                ps_v = proj("wv")
                nc.vector.tensor_copy(vT[:, sl], ps_v[:])

            # ---- phase A2: build [V | 1] stationary tiles (transpose v) ----
            nc.vector.memset(vext[:], 1.0)
            for h in range(HPC):
                for i in range(NKT):
                    trp = scp.tile([128, SQC], F32, tag="scps")
                    nc.tensor.transpose(
                        trp[:, 0:64],
                        vT[h * 64 : (h + 1) * 64, i * 128 : (i + 1) * 128],
                        eye_sb[h * 64 : (h + 1) * 64, :],
                    )
                    base = (h * NKT + i) * 65
                    nc.vector.tensor_copy(vext[:, base : base + 64], trp[:, 0:64])

            # ---- phase B: scores -> softmax -> attn@V -> O-proj ----
            for j in range(NJ):
                sl = slice(j * SQC, (j + 1) * SQC)
                nkt_j = 4 * (j + 1) if mode == "causal" else NKT
                afin = afp.tile([128, SQC], mm_dt, tag="afin")
                at_t0 = at0p.tile([65, SQC], F32, tag="at0")
                at_t1 = at1p.tile([65, SQC], F32, tag="at1")
                atp = [at_t0, at_t1]
                for i in range(nkt_j):
                    msk = None
                    if mode == "causal" and i >= 4 * j:
                        msk = mkp.tile([128, SQC], F32, tag="msk")
                        r = (i - 4 * j) * 128
                        nc.sync.dma_start(msk[:], mask_d[j, r : r + 128, :])
                    elif mode == "general":
                        msk = mkp.tile([128, SQC], F32, tag="msk")
                        nc.sync.dma_start(
                            msk[:], mask_d[j, i * 128 : (i + 1) * 128, :]
                        )
                    for h in range(HPC):
                        hsl = slice(h * 64, (h + 1) * 64)
                        sps = scp.tile([128, SQC], F32, tag="scps")
                        nc.tensor.matmul(
                            sps[:],
                            krot[hsl, i * 128 : (i + 1) * 128],
                            qrot[hsl, sl],
                            start=True,
                            stop=True,
                        )
                        if msk is not None:
                            nc.vector.tensor_add(sps[:], sps[:], msk[:])
                        ex = exp_p.tile([128, SQC], mm_dt, tag="ex")
                        nc.scalar.activation(ex[:], sps[:], AF.Exp, scale=0.125)
                        vbase = (h * NKT + i) * 65
                        nc.tensor.matmul(
                            atp[h][:, :],
                            vext[:, vbase : vbase + 65],
                            ex[:],
                            start=(i == 0),
                            stop=(i == nkt_j - 1),
                        )
                # normalize: rows 0:64 are attn@V, row 64 is the denominator
                for h in range(HPC):
                    rec = rcp.tile([128, SQC], F32, tag="rec")
                    nc.vector.reciprocal(rec[64:65, :], atp[h][64:65, :])
                    rec0 = rcp.tile([1, SQC], F32, tag="rec0")
                    nc.sync.dma_start(rec0[:], rec[64:65, :])
                    bcps = scp.tile([128, SQC], F32, tag="scps")
                    nc.tensor.matmul(
                        bcps[0:64, :], ones_sb[:], rec0[:], start=True, stop=True
                    )
                    bcs = bcp_p.tile([64, SQC], F32, tag="bcs")
                    nc.vector.tensor_copy(bcs[:], bcps[0:64, :])
                    if h == 0:
                        nc.vector.tensor_mul(afin[0:64, :], atp[0][0:64, :], bcs[:])
                    else:
                        tmph = tmp_p.tile([64, SQC], mm_dt, tag="tmph")
                        nc.vector.tensor_mul(tmph[:], atp[1][0:64, :], bcs[:])
                        nc.sync.dma_start(afin[64:128, :], tmph[:])
                # output projection: partial.T[dout, sq]
                for dt_i in range(KT):
                    op = ppp.tile([128, SQC], F32, tag="pp")
                    nc.tensor.matmul(
                        op[:],
                        wo_sb[:, dt_i * 128 : (dt_i + 1) * 128],
                        afin[:],
                        start=True,
                        stop=True,
                    )
                    os_t = oop.tile([128, SQC], F32, tag="oo")
                    nc.vector.tensor_copy(os_t[:], op[:])
                    nc.sync.dma_start(
                        out_d[dt_i * 128 : (dt_i + 1) * 128, sl], os_t[:]
                    )

    _fix_waits(nc)
    return nc


def _host_prep(x, cos, sin, mask, wq, wk, wv, wo):
    xT = np.ascontiguousarray(x.reshape(S, D).T).astype(np.float32)

    idx = np.repeat(np.arange(HD // 2), 2)
    cosP_h = np.ascontiguousarray(np.asarray(cos)[:, idx].T)  # (64, S)
    sinP_h = np.ascontiguousarray(np.asarray(sin)[:, idx].T)
    cosP = np.vstack([cosP_h, cosP_h]).astype(np.float32)
    sinP = np.vstack([sinP_h, sinP_h]).astype(np.float32)

    eye2 = np.vstack([np.eye(64), np.eye(64)]).astype(np.float32)

    mask = np.asarray(mask)
    neg = np.isneginf(mask)
    triu = np.triu(np.ones((S, S), dtype=bool), 1)
    if not neg.any() and not mask.any():
        mode = "zeros"
        maskd = None
    elif np.array_equal(neg, triu) and not mask[~neg].any():
        mode = "causal"
        maskd = np.empty((NJ, SQC, SQC), np.float32)
        for j in range(NJ):
            blk = mask[j * SQC : (j + 1) * SQC, j * SQC : (j + 1) * SQC].T
            maskd[j] = blk * np.float32(8.0)
    else:
        mode = "general"
        maskd = np.empty((NJ, S, SQC), np.float32)
        for j in range(NJ):
            maskd[j] = mask[j * SQC : (j + 1) * SQC, :].T * np.float32(8.0)

    per_core = []
    for c in range(NC):
        hs, he = c * 128, (c + 1) * 128
        m = {
            "xT": xT,
            "cosP": cosP,
            "sinP": sinP,
            "eye2": eye2,
            "wo": np.ascontiguousarray(np.asarray(wo)[:, hs:he].T).astype(np.float32),
        }
        for name, w in (("wq", wq), ("wk", wk)):
            ws = np.asarray(w)[hs:he, :].astype(np.float32)
            sw = np.empty_like(ws)
            sw[0::2] = -ws[1::2]
            sw[1::2] = ws[0::2]
            m[name] = np.ascontiguousarray(ws.T)
            m[name + "s"] = np.ascontiguousarray(sw.T)
        m["wv"] = np.ascontiguousarray(np.asarray(wv)[hs:he, :].T).astype(np.float32)
        if maskd is not None:
            m["maskd"] = maskd
        per_core.append(m)
    return mode, per_core


_cache = {}


def kernel(x, cos, sin, mask, wq, wk, wv, wo, start_pos=0, **_):
    mode, in_maps = _host_prep(
        np.asarray(x), cos, sin, mask, np.asarray(wq), np.asarray(wk),
        np.asarray(wv), np.asarray(wo)
    )
    if mode not in _cache:
        _cache[mode] = build_program(mode)
    nc = _cache[mode]
    res = run_bass_kernel_spmd(nc, in_maps, core_ids=list(range(NC)))
    acc = np.zeros((D, S), np.float64)
    for c in range(NC):
        acc += res.results[c]["opT"].astype(np.float64)
    return np.ascontiguousarray(acc.T).reshape(B, S, D).astype(np.float32)
